# revision 14
# baseline (speedup 1.0000x reference)
"""DiceLoss (CondInst-style dynamic mask head) Trainium2 kernel.

Data-parallel over batch B=8: one image per NeuronCore. Per image:
  - gather per-object 1x1-conv weights from conv_weight at ind (host, tiny)
  - conv1: [10 -> 8] per object (relu), conv2: [8 -> 8] (relu),
    conv3: [8 -> 1] (sigmoid) over all HW=16384 pixels, K=32 objects
  - dice partial sums sum(p*t), sum(p*p), sum(t*t) per image
Host folds the relative-coordinate channels into conv1's bias (they are
affine in the pixel grid), builds block-diagonal weights so all 32 objects
run as dense 128-contraction matmuls, pre-masks target, and forces
sigmoid->0 for masked objects via a large negative conv3 bias.

Device layout (per core):
  conv1/conv2 over two halves of 16 objects: M=128=(k16,o8), N=512 chunks.
  conv3 packs 4 hw-chunks into one PSUM bank via col-tiling (M=32 at
  partition 32j), so sigmoid+dice run on full-128-partition tiles.
"""

import numpy as np
import ml_dtypes

import concourse.bass as bass
import concourse.mybir as mybir
import concourse.tile as tile
from concourse.bass_utils import run_bass_kernel_spmd

BF16 = mybir.dt.bfloat16
F32 = mybir.dt.float32

B, C, K, H, W = 8, 8, 32, 128, 128
HW = H * W
CW = 169
NCHUNK = 32          # hw chunks of 512
CHUNK = 512
NGROUP = 8           # col-tile groups of 4 chunks
N_CORES = 8

_NEG_BIG = 30000.0   # sigmoid(-30000 + z) == 0 for any realistic z


# ---------------------------------------------------------------------------
# The bundled neuronxcc rejects Tile's kernel-tail drain when it carries >2
# sem waits (CTRL encoding limit in CoreV3 codegen). Split the waits into
# individual wait_ge nops on the sync sequencer (semantically equivalent:
# the sequencer executes them serially before the drain's barrier).
# ---------------------------------------------------------------------------
def _drain_and_barrier_split(self, tick_clock, wait_clock):
    from concourse.tile import ScopedClock

    nc = self.nc
    drain_inst = nc.sync.drain()
    wait_clock.add_sem_waits(
        drain_inst.ins, ScopedClock({None: tick_clock.global_clock})
    )
    si = drain_inst.ins.sync_info
    waits = list(si.on_wait) if si is not None else []
    if len(waits) > 1:
        drain_inst.ins.sync_info = None
        handles = list(self.sems.allocated().values())
        by_num = {h.num: h for h in handles}
        by_name = {h.name: h for h in handles}
        for w_ in waits:
            h = by_num.get(w_.id) or by_name.get(w_.ant_name)
            assert h is not None, f"no semaphore handle for {w_}"
            assert w_.wait_mode == "sem-ge-imm", w_.wait_mode
            nc.sync.wait_ge(h, w_.wait_value)
    nc.all_engine_barrier()
    popped = nc._tile_sem_poison_stack.pop()
    assert popped is self._sem_poison
    nc.clear_and_free_semaphores(list(self.sems.allocated().values()))
    nc.all_engine_barrier()


tile.TileContext._drain_and_barrier = _drain_and_barrier_split


def _wait_limit(inst) -> int:
    # Universal: this walrus build's encodings reliably hold only one
    # sem wait (TensorScalarPtr/Activation/Drain all reject 2+).
    return 1


def split_excess_waits(nc, register=True):
    """Walrus codegen caps sem waits per instruction (2 generally, 1 for
    pointer-operand encodings). Spill excess waits onto NoOps inserted just
    before the instruction in the same engine's stream."""
    for f in nc.m.functions:
        for bb in f.blocks:
            out = []
            changed = False
            for inst in bb.instructions:
                si = inst.sync_info
                waits = list(si.on_wait) if si is not None else []
                limit = _wait_limit(inst)
                if len(waits) > limit:
                    keep = waits[:limit]
                    spill = waits[limit:]
                    for i, w_ in enumerate(spill):
                        nop = mybir.InstNoOp(
                            name=f"{inst.name}_wspill{i}",
                            engine=inst.engine,
                            sync_info=mybir.SyncInfo(on_wait=[w_], on_update=[]),
                            bass_nofuse=True,
                        )
                        if register:
                            nc.register_instruction(nop, overwrite=True)
                        out.append(nop)
                    inst.sync_info = mybir.SyncInfo(
                        on_wait=keep, on_update=list(si.on_update)
                    )
                    changed = True
                out.append(inst)
            if changed:
                bb.instructions = out


# ---------------------------------------------------------------------------
# Device kernel
# ---------------------------------------------------------------------------
def build_nc():
    nc = bass.Bass()
    f10_d = nc.declare_dram_parameter("f10", [10, HW], BF16, False)
    w1t_d = nc.declare_dram_parameter("w1t", [10, 256], BF16, False)
    w2t_d = nc.declare_dram_parameter("w2t", [128, 256], BF16, False)
    w3t_d = nc.declare_dram_parameter("w3t", [128, 64], BF16, False)
    b12_d = nc.declare_dram_parameter("b12", [128, 4], F32, False)
    b3_d = nc.declare_dram_parameter("b3", [128, 1], F32, False)
    tpk_d = nc.declare_dram_parameter("tpk", [128, 4096], BF16, False)
    sums_d = nc.declare_dram_parameter("sums", [128, 16], F32, True)
    ptsum_d = nc.declare_dram_parameter("ptsum", [1, 512], F32, True)

    RELU = mybir.ActivationFunctionType.Relu
    SIGM = mybir.ActivationFunctionType.Sigmoid
    SQUARE = mybir.ActivationFunctionType.Square
    ADD = mybir.AluOpType.add
    MAX = mybir.AluOpType.max
    MULT = mybir.AluOpType.mult

    with tile.TileContext(nc) as tc:
        with (
            tc.tile_pool(name="const", bufs=1) as const,
            tc.tile_pool(name="work", bufs=3) as work,
            tc.tile_pool(name="predp", bufs=2) as predp,
            tc.tile_pool(name="scratch", bufs=2) as scratch,
            tc.tile_pool(name="ps1", bufs=2, space="PSUM") as ps1p,
            tc.tile_pool(name="ps2", bufs=3, space="PSUM") as ps2p,
            tc.tile_pool(name="ps3", bufs=2, space="PSUM") as ps3p,
            tc.tile_pool(name="psred", bufs=1, space="PSUM") as psredp,
        ):
            w1_sb = const.tile([10, 256], BF16)
            nc.gpsimd.dma_start(out=w1_sb[:], in_=w1t_d[:])
            w2_sb = const.tile([128, 256], BF16)
            nc.gpsimd.dma_start(out=w2_sb[:], in_=w2t_d[:])
            w3_sb = const.tile([128, 64], BF16)
            nc.gpsimd.dma_start(out=w3_sb[:], in_=w3t_d[:])
            b12_sb = const.tile([128, 4], F32)
            nc.gpsimd.dma_start(out=b12_sb[:], in_=b12_d[:])
            b3_sb = const.tile([128, 1], F32)
            nc.gpsimd.dma_start(out=b3_sb[:], in_=b3_d[:])
            f_sb = const.tile([10, HW], BF16)
            nc.gpsimd.dma_start(out=f_sb[:], in_=f10_d[:])
            tpk_sb = const.tile([128, 4096], BF16)
            nc.gpsimd.dma_start(out=tpk_sb[:], in_=tpk_d[:])

            # per-partition dice accumulators: cols 0:8 pp, 8:16 tt
            accs = const.tile([128, 16], F32)
            ones_sb = const.tile([128, 1], BF16)
            nc.vector.memset(ones_sb, 1.0)
            red_pt = psredp.tile([1, CHUNK], F32)

            def evac_relu(dst, src, bias_ap, on_act):
                if on_act:
                    nc.scalar.activation(
                        out=dst[:], in_=src[:], func=RELU, bias=bias_ap
                    )
                else:
                    nc.vector.tensor_scalar(
                        out=dst[:],
                        in0=src[:],
                        scalar1=bias_ap,
                        scalar2=0.0,
                        op0=ADD,
                        op1=MAX,
                    )

            for d in range(NGROUP):
                ps3 = ps3p.tile([128, CHUNK], F32, tag="ps3")
                for j in range(4):
                    c = 4 * d + j
                    cs = bass.ts(c, CHUNK)
                    ps1a = ps1p.tile([128, CHUNK], F32, tag="ps1")
                    nc.tensor.matmul(
                        ps1a[:], w1_sb[:, 0:128], f_sb[:, cs], start=True, stop=True
                    )
                    h1a = work.tile([128, CHUNK], BF16, tag="h1a")
                    evac_relu(h1a, ps1a, b12_sb[:, 0:1], on_act=False)

                    ps1b = ps1p.tile([128, CHUNK], F32, tag="ps1")
                    nc.tensor.matmul(
                        ps1b[:], w1_sb[:, 128:256], f_sb[:, cs], start=True, stop=True
                    )
                    h1b = work.tile([128, CHUNK], BF16, tag="h1b")
                    evac_relu(h1b, ps1b, b12_sb[:, 1:2], on_act=True)

                    ps2a = ps2p.tile([128, CHUNK], F32, tag="ps2")
                    nc.tensor.matmul(
                        ps2a[:], w2_sb[:, 0:128], h1a[:], start=True, stop=True
                    )
                    h2a = work.tile([128, CHUNK], BF16, tag="h2a")
                    evac_relu(h2a, ps2a, b12_sb[:, 2:3], on_act=True)

                    ps2b = ps2p.tile([128, CHUNK], F32, tag="ps2")
                    nc.tensor.matmul(
                        ps2b[:], w2_sb[:, 128:256], h1b[:], start=True, stop=True
                    )
                    h2b = work.tile([128, CHUNK], BF16, tag="h2b")
                    evac_relu(h2b, ps2b, b12_sb[:, 3:4], on_act=(j % 4 == 0))

                    # conv3: both halves accumulate into partitions 32j:32j+32
                    nc.tensor.matmul(
                        ps3[32 * j : 32 * j + 32, :],
                        w3_sb[:, 0:32],
                        h2a[:],
                        start=True,
                        stop=False,
                        tile_position=(0, 32 * j),
                    )
                    nc.tensor.matmul(
                        ps3[32 * j : 32 * j + 32, :],
                        w3_sb[:, 32:64],
                        h2b[:],
                        start=False,
                        stop=True,
                        tile_position=(0, 32 * j),
                    )

                pred = predp.tile([128, CHUNK], BF16, tag="pred")
                nc.scalar.activation(
                    out=pred[:], in_=ps3[:], func=SIGM, bias=b3_sb[:, 0:1]
                )
                tgt = tpk_sb[:, bass.ts(d, CHUNK)]
                # pt: DVE product, summed over hw by PE (ones-matmul)
                pt_s = scratch.tile([128, CHUNK], BF16, tag="pt_s")
                nc.vector.tensor_mul(out=pt_s[:], in0=pred[:], in1=tgt)
                nc.tensor.matmul(
                    red_pt[:], ones_sb[:], pt_s[:],
                    start=(d == 0), stop=(d == NGROUP - 1),
                )
                # pp / tt: ACT square with fused free-dim accumulate
                pp_s = scratch.tile([128, CHUNK], BF16, tag="pp_s")
                nc.scalar.activation(
                    out=pp_s[:], in_=pred[:], func=SQUARE,
                    accum_out=accs[:, d : d + 1],
                )
                tt_s = scratch.tile([128, CHUNK], BF16, tag="tt_s")
                nc.scalar.activation(
                    out=tt_s[:], in_=tgt, func=SQUARE,
                    accum_out=accs[:, 8 + d : 9 + d],
                )

            ptsum_sb = const.tile([1, CHUNK], F32)
            nc.scalar.copy(out=ptsum_sb[:], in_=red_pt[:])
            nc.gpsimd.dma_start(out=ptsum_d[:], in_=ptsum_sb[:])
            nc.gpsimd.dma_start(out=sums_d[:], in_=accs[:])
    split_excess_waits(nc)
    return nc


# ---------------------------------------------------------------------------
# Host-side input preparation (numpy, per image)
# ---------------------------------------------------------------------------
def prep_inputs(seg_feat, conv_weight, mask, ind, target):
    seg_feat = np.asarray(seg_feat)
    conv_weight = np.asarray(conv_weight)
    mask = np.asarray(mask)
    ind = np.asarray(ind).astype(np.int64)
    target = np.asarray(target)

    cw = conv_weight.reshape(B, CW, HW)
    w = np.take_along_axis(cw, ind[:, None, :], axis=2)  # [B, CW, K]
    w = np.ascontiguousarray(w.transpose(0, 2, 1)).astype(np.float32)  # [B,K,CW]

    c1w = w[..., 0:80].reshape(B, K, C, C + 2)       # [B,K,8,10]
    c1b = w[..., 80:88]                              # [B,K,8]
    c2w = w[..., 88:152].reshape(B, K, C, C)         # [B,K,8,8]
    c2b = w[..., 152:160]                            # [B,K,8]
    c3w = w[..., 160:168].reshape(B, K, C)           # [B,K,8]
    c3b = w[..., 168]                                # [B,K]

    x = (ind % W).astype(np.float32) / W             # [B,K]
    y = (ind // W).astype(np.float32) / H
    b1eff = c1b - c1w[..., 8] * x[:, :, None] - c1w[..., 9] * y[:, :, None]

    mf = mask.astype(np.float32)                     # [B,K]
    b3eff = c3b - _NEG_BIG * (1.0 - mf)              # [B,K]

    xg = (np.arange(HW, dtype=np.float32) % W) / W
    yg = (np.arange(HW, dtype=np.float32) // W) / H

    bf = ml_dtypes.bfloat16
    in_maps = []
    for b in range(B):
        f10 = np.concatenate(
            [seg_feat[b].reshape(C, HW), xg[None], yg[None]], axis=0
        ).astype(bf)

        w1t = np.ascontiguousarray(
            c1w[b].transpose(2, 0, 1).reshape(C + 2, K * C)
        ).astype(bf)  # [10, 256]

        w2t = np.zeros((128, 256), np.float32)
        for half in range(2):
            for kl in range(16):
                blk = c2w[b, half * 16 + kl].T  # [c, o]
                w2t[kl * 8 : kl * 8 + 8, half * 128 + kl * 8 : half * 128 + kl * 8 + 8] = blk
        w2t = w2t.astype(bf)

        w3t = np.zeros((128, 64), np.float32)
        for half in range(2):
            for kl in range(16):
                kk = half * 16 + kl
                w3t[kl * 8 : kl * 8 + 8, half * 32 + kk] = c3w[b, kk]
        w3t = w3t.astype(bf)

        b12 = np.stack(
            [
                b1eff[b].reshape(K * C)[0:128],
                b1eff[b].reshape(K * C)[128:256],
                c2b[b].reshape(K * C)[0:128],
                c2b[b].reshape(K * C)[128:256],
            ],
            axis=1,
        ).astype(np.float32)  # [128, 4]

        b3 = np.tile(b3eff[b], 4)[:, None].astype(np.float32)  # [128, 1]

        t_m = (target[b] * mf[b][:, None, None]).reshape(K, HW)
        tpk = np.ascontiguousarray(
            t_m.reshape(K, 8, 4, CHUNK).transpose(2, 0, 1, 3).reshape(128, 4096)
        ).astype(bf)

        in_maps.append(
            {
                "f10": f10,
                "w1t": w1t,
                "w2t": w2t,
                "w3t": w3t,
                "b12": b12,
                "b3": b3,
                "tpk": tpk,
            }
        )
    return in_maps


def finish(outs_list):
    per_img = np.empty(B, np.float64)
    for b in range(B):
        sums, ptsum = outs_list[b]
        s = np.asarray(sums, np.float64)  # [128, 16]
        inter = np.asarray(ptsum, np.float64).sum()
        spp = s[:, 0:8].sum()
        stt = s[:, 8:16].sum()
        per_img[b] = 1.0 - (2.0 * inter + 1.0) / (spp + stt + 1.0)
    return np.float32(per_img.mean())


_NC_CACHE = {}


def kernel(seg_feat, conv_weight, mask, ind, target):
    if "nc" not in _NC_CACHE:
        _NC_CACHE["nc"] = build_nc()
    nc = _NC_CACHE["nc"]
    in_maps = prep_inputs(seg_feat, conv_weight, mask, ind, target)
    res = run_bass_kernel_spmd(nc, in_maps, list(range(N_CORES)))
    return finish(
        [(res.results[b]["sums"], res.results[b]["ptsum"]) for b in range(B)]
    )
